# revision 34
# baseline (speedup 1.0000x reference)
"""Routed quantized MoE eval kernel for 8 Trainium2 NeuronCores.

Strategy (expert-parallel with true top-2 routing):
- Core c owns expert e=c. Expert weights are dequantized (scale-folded),
  cast to fp16 and transposed on the host at shard-prep time.
- Router runs dense on-device per 512-token chunk with split-precision
  fp16 matmuls (x16@[rw_hi|rw_lo] packed in one stationary + xres@rw_hi)
  accumulated in fp32 PSUM; top-2 + softmax + combine weights on DVE.
- ROUTED expert compute: for each chunk, tokens whose top-2 includes
  this core's expert are compacted via a PE-cumsum rank (triangular-ones
  matmul), their (token id, ca*alpha) pairs scattered into a DRAM slot
  table by indirect DMA (pads stay OOB), the x rows gathered by indirect
  DMA from a row-major fp16 copy of x, PE-transposed, and only those
  <=256 tokens (of 512) run the expert SwiGLU matmuls.
- Shared MLP sharded along DF_S (each core computes 256 of 2048 ffn
  rows) runs dense; each chunk's base rows (1-s)*shared_partial are
  written to a DRAM buffer, expert rows ca*alpha*eo are indirect-DMA
  scatter-ADDed on top (OOB pad slots skipped), then a chunked
  ReduceScatter sums across the 8 cores.
- Routing metadata for chunk c+1 is computed inside chunk c's body so
  all gathers complete before their expert matmuls need them.

Output identity:
  mixed = (1 - sum_e ca_e*alpha_e) * shared + sum_e ca_e*alpha_e * eo_e
"""

import numpy as np
from contextlib import ExitStack

import concourse.bass as bass
import concourse.tile as tile
from concourse import bacc, mybir
from concourse.bass_utils import run_bass_kernel_spmd

NCORES = 8
B, S, D = 2, 1024, 1024
T = B * S                      # 2048 tokens
DF_E, DF_S, E = 512, 2048, 8
FS = DF_S // NCORES            # 256 shared-ffn rows per core
CH = 4                         # token chunks
CT = T // CH                   # 512 tokens per chunk
TT = CT // 128                 # 4 token tiles per chunk
KD = D // 128                  # 8 k-tiles over hidden dim
KF = DF_E // 128               # 4 k-tiles over expert ffn dim
KS = FS // 128                 # 2 k-tiles over shared ffn shard
ND = D // 512                  # 2 output column slices
CAP = 192                      # routed-token capacity per (expert, chunk)
CAPM = 256                     # slot-metadata width (full 2x128)
NST = 2                        # slot tiles per chunk
BIGF = 4.0e6                   # OOB pusher for pad slots / ids

F16 = mybir.dt.float16
F32 = mybir.dt.float32
I32 = mybir.dt.int32
ACTF = mybir.ActivationFunctionType
ALU = mybir.AluOpType

_CACHE = {}


def _build():
    nc = bacc.Bacc(
        "TRN2", target_bir_lowering=False, debug=False, num_devices=NCORES
    )

    xT = nc.dram_tensor("xT", [D, T], F16, kind="ExternalInput").ap()
    xR = nc.dram_tensor("xR", [D, T], F16, kind="ExternalInput").ap()
    xrows = nc.dram_tensor("xrows", [T, D], F16, kind="ExternalInput").ap()
    # router weights split-precision: cols 0:E fp16-hi, E:2E fp16 residual
    rwP = nc.dram_tensor("rwP", [D, 2 * E], F16, kind="ExternalInput").ap()
    gqT = nc.dram_tensor("gqT", [D, DF_E], F16, kind="ExternalInput").ap()
    uqT = nc.dram_tensor("uqT", [D, DF_E], F16, kind="ExternalInput").ap()
    dqT = nc.dram_tensor("dqT", [DF_E, D], F16, kind="ExternalInput").ap()
    wgT = nc.dram_tensor("wgT", [D, FS], F16, kind="ExternalInput").ap()
    wuT = nc.dram_tensor("wuT", [D, FS], F16, kind="ExternalInput").ap()
    wdT = nc.dram_tensor("wdT", [FS, D], F16, kind="ExternalInput").ap()
    # aux[:, 0:8] = alpha broadcast, aux[:, 8:16] = onehot(expert) broadcast
    aux = nc.dram_tensor("aux", [128, 2 * E], F32, kind="ExternalInput").ap()
    OUT = nc.dram_tensor("OUT", [4 * 64, D], F16, kind="ExternalOutput").ap()

    with ExitStack() as ctx:
        tc = ctx.enter_context(tile.TileContext(nc))
        wres = ctx.enter_context(tc.tile_pool(name="wres", bufs=1))
        xs = ctx.enter_context(tc.tile_pool(name="xs", bufs=1))
        xg = ctx.enter_context(tc.tile_pool(name="xg", bufs=3))
        hp = ctx.enter_context(tc.tile_pool(name="hp", bufs=2))
        work = ctx.enter_context(tc.tile_pool(name="work", bufs=2))
        rt = ctx.enter_context(tc.tile_pool(name="rt", bufs=2))
        meta = ctx.enter_context(tc.tile_pool(name="meta", bufs=2))
        ps_gu = ctx.enter_context(tc.tile_pool(name="ps_gu", bufs=3, space="PSUM"))
        ps_dn = ctx.enter_context(tc.tile_pool(name="ps_dn", bufs=3, space="PSUM"))
        ps_r = ctx.enter_context(tc.tile_pool(name="ps_r", bufs=2, space="PSUM"))
        dram = ctx.enter_context(tc.tile_pool(name="dram", bufs=1, space="DRAM"))

        def load_rows(src, rows, cols, name):
            tiles = []
            r = src.rearrange("(k p) n -> k p n", p=128)
            for k in range(rows // 128):
                t = wres.tile([128, cols], src.dtype, tag=f"{name}{k}")
                nc.sync.dma_start(t[:], r[k])
                tiles.append(t)
            return tiles

        from concourse.masks import make_identity, make_upper_triangular

        # warmup ReduceScatter FIRST on the in-order gpsimd queue: the
        # ~54us ncfw cold-start begins at trigger time, so trigger ASAP
        wu_in = dram.tile([NCORES, 128], F16, tag="wuin")
        wu_out = dram.tile([1, 128], F16, tag="wuout")
        nc.sync.dma_start(wu_in[:], gqT[0:NCORES, 0:128])
        nc.gpsimd.collective_compute(
            "ReduceScatter",
            ALU.add,
            replica_groups=[list(range(NCORES))],
            ins=[wu_in.opt()],
            outs=[wu_out.opt()],
        )
        # second tiny RS fully warms the ncfw pipeline during the CC-idle
        # window so the real per-chunk RS ops run at warm speed
        wu_out2 = dram.tile([1, 128], F16, tag="wuout2")
        nc.gpsimd.collective_compute(
            "ReduceScatter",
            ALU.add,
            replica_groups=[list(range(NCORES))],
            ins=[wu_in.opt()],
            outs=[wu_out2.opt()],
        )
        ident = wres.tile([128, 128], F32, tag="ident")
        make_identity(nc, ident[:])
        ident16 = wres.tile([128, 128], F16, tag="ident16")
        make_identity(nc, ident16[:])
        # inclusive upper-triangular ones (lhsT for partition cumsum)
        utri = wres.tile([128, 128], F16, tag="utri")
        make_upper_triangular(nc, utri[:], val=1.0, diag=True)
        ones128 = wres.tile([128, 128], F16, tag="ones128")
        nc.gpsimd.memset(ones128[:], 1.0)
        # iota4[p, j] = j*128 + p  (local token id within a chunk)
        iota4_i = wres.tile([128, TT], I32, tag="iota4i")
        nc.gpsimd.iota(iota4_i[:], pattern=[[128, TT]], base=0,
                       channel_multiplier=1)
        iota4_f = wres.tile([128, TT], F32, tag="iota4f")
        nc.vector.tensor_copy(iota4_f[:], iota4_i[:])
        # iota256[p, s] = s (slot index along free axis)
        iota256_i = wres.tile([128, CAPM], I32, tag="iota256i")
        nc.gpsimd.iota(iota256_i[:], pattern=[[1, CAPM]], base=0,
                       channel_multiplier=0)
        iota256_f = wres.tile([128, CAPM], F32, tag="iota256f")
        nc.vector.tensor_copy(iota256_f[:], iota256_i[:, 0:CAPM])

        # dummy matmuls during the initial DMA wait: warms the PE clock
        wu_ps = ps_r.tile([128, 128], F32, tag="psx", bufs=2)
        for _ in range(36):
            nc.tensor.matmul(wu_ps[:], ident[:], ident[:], start=True, stop=True)

        rw = load_rows(rwP, D, 2 * E, "rw")
        aux_sb = wres.tile([128, 2 * E], F32, tag="aux")
        nc.sync.dma_start(aux_sb[:], aux[:])
        alpha_bc = aux_sb[:, 0:E]
        sel_bc = aux_sb[:, E : 2 * E]

        xTr = xT.rearrange("(k p) t -> k p t", p=128)
        xRr = xR.rearrange("(k p) t -> k p t", p=128)

        xt_all = [None] * CH
        xr_all = [None] * CH

        def load_x(c):
            xt, xr = [], []
            for k in range(KD):
                t = xs.tile([128, CT], F16, tag=f"xt{c}_{k}")
                nc.sync.dma_start(t[:], xTr[k, :, c * CT : (c + 1) * CT])
                xt.append(t)
            for k in range(KD):
                t = xs.tile([128, CT], F16, tag=f"xr{c}_{k}")
                nc.sync.dma_start(t[:], xRr[k, :, c * CT : (c + 1) * CT])
                xr.append(t)
            xt_all[c], xr_all[c] = xt, xr

        load_x(0)
        wg = load_rows(wgT, D, FS, "wg")
        wu = load_rows(wuT, D, FS, "wu")
        load_x(1)
        load_x(2)
        gq = load_rows(gqT, D, DF_E, "gq")
        uq = load_rows(uqT, D, DF_E, "uq")
        wd = load_rows(wdT, FS, D, "wd")
        dq = load_rows(dqT, DF_E, D, "dq")

        # per-chunk routing state, filled by route_*()
        st_oneminus = [None] * CH   # [128,1] slices per j: (1 - s)
        st_smeta = [None] * CH      # list per st: (gid_i, lid_i, ca_slot)
        st_xgrows = [None] * CH     # gathered x rows tiles per st
        st_xgT = [None] * CH        # transposed gathered x per k-tile

        def route_mms(c):
            """Router matmuls for chunk c -> Lt [E, CT] f32 in SBUF."""
            xt, xr = xt_all[c], xr_all[c]
            ps_lg = ps_r.tile([40, CT], F32, tag="pslog", bufs=1, name="ps_lg")
            ps_lt = ps_lg[0 : 2 * E, :]
            ps_lr = ps_lg[32:40, :]
            for k in range(KD):
                nc.tensor.matmul(
                    ps_lt, rw[k][:, 0 : 2 * E], xt[k][:],
                    start=(k == 0), stop=(k == KD - 1),
                )
            for k in range(KD):
                nc.tensor.matmul(
                    ps_lr, rw[k][:, 0:E], xr[k][:],
                    start=(k == 0), stop=(k == KD - 1),
                )
            LtA = rt.tile([2 * E, CT], F32, tag="LtA")
            nc.vector.tensor_copy(LtA[:], ps_lg[0 : 2 * E, :])
            LtB = rt.tile([E, CT], F32, tag="LtB")
            nc.vector.tensor_copy(LtB[:], ps_lr)
            return (LtA, LtB)

        def route_combine(c, Lt):
            """Transpose logits, top-2 softmax, combine weights, and the
            rank/slot computation feeding the routed gather."""
            LtA, LtB = Lt
            ps_lx = ps_r.tile([128, 128], F32, tag="psx", bufs=2, name="ps_lx")
            ps_l = ps_lx[:, 0 : TT * 3 * E]
            for j in range(TT):
                nc.tensor.transpose(
                    ps_l[:, j * 3 * E : j * 3 * E + 2 * E],
                    LtA[:, j * 128 : (j + 1) * 128],
                    ident[0 : 2 * E, 0 : 2 * E],
                )
                nc.tensor.transpose(
                    ps_l[:, j * 3 * E + 2 * E : (j + 1) * 3 * E],
                    LtB[:, j * 128 : (j + 1) * 128],
                    ident[0:E, 0:E],
                )
            Lall = rt.tile([128, TT * 3 * E], F32, tag="Lall")
            nc.vector.tensor_copy(Lall[:], ps_l[:])
            va = Lall[:].rearrange("p (j x) -> p j x", x=3 * E)
            L = rt.tile([128, TT * E], F32, tag="L")
            L3 = L[:].rearrange("p (j e) -> p j e", e=E)
            nc.vector.tensor_tensor(L3, va[:, :, 0:E], va[:, :, E : 2 * E],
                                    op=ALU.add)
            nc.vector.tensor_tensor(L3, L3, va[:, :, 2 * E : 3 * E],
                                    op=ALU.add)

            def bc(t):  # [128, TT] -> [128, TT, E] free-axis broadcast
                return t[:, :, None].broadcast_to([128, TT, E])

            m1 = rt.tile([128, TT], F32, tag="m1")
            nc.vector.tensor_reduce(m1[:], L3, mybir.AxisListType.X, ALU.max)
            mask1 = rt.tile([128, TT * E], F32, tag="mask1")
            mask1_3 = mask1[:].rearrange("p (j e) -> p j e", e=E)
            nc.vector.tensor_tensor(mask1_3, L3, bc(m1), op=ALU.is_ge)
            L2 = rt.tile([128, TT * E], F32, tag="L2")
            nc.vector.scalar_tensor_tensor(
                L2[:], mask1[:], -1e30, L[:], ALU.mult, ALU.add
            )
            L2_3 = L2[:].rearrange("p (j e) -> p j e", e=E)
            m2 = rt.tile([128, TT], F32, tag="m2")
            nc.vector.tensor_reduce(m2[:], L2_3, mybir.AxisListType.X, ALU.max)
            mask2 = rt.tile([128, TT * E], F32, tag="mask2")
            mask2_3 = mask2[:].rearrange("p (j e) -> p j e", e=E)
            nc.vector.tensor_tensor(mask2_3, L2_3, bc(m2), op=ALU.is_ge)
            # softmax over {m1, m2}: w1 = sigmoid(m1 - m2), w2 = 1 - w1
            dlt = rt.tile([128, TT], F32, tag="dlt")
            nc.vector.tensor_sub(dlt[:], m1[:], m2[:])
            w1 = rt.tile([128, TT], F32, tag="w1")
            nc.scalar.activation(w1[:], dlt[:], ACTF.Sigmoid)
            w2 = rt.tile([128, TT], F32, tag="w2")
            nc.vector.tensor_scalar(w2[:], w1[:], -1.0, 1.0, ALU.mult, ALU.add)
            caw = rt.tile([128, TT * E], F32, tag="caw")
            caw3 = caw[:].rearrange("p (j e) -> p j e", e=E)
            nc.vector.tensor_tensor(caw3, mask2_3, bc(w2), op=ALU.mult)
            t1 = rt.tile([128, TT * E], F32, tag="t1")
            t1_3 = t1[:].rearrange("p (j e) -> p j e", e=E)
            nc.vector.tensor_tensor(t1_3, mask1_3, bc(w1), op=ALU.mult)
            nc.vector.tensor_add(caw[:], caw[:], t1[:])
            # scale by alpha (broadcast over token-tiles) and reduce
            ca_a = rt.tile([128, TT * E], F32, tag="ca_a")
            ca_a3 = ca_a[:].rearrange("p (j e) -> p j e", e=E)
            alpha3 = alpha_bc[:, None, :].broadcast_to([128, TT, E])
            nc.vector.tensor_tensor(ca_a3, caw3, alpha3, op=ALU.mult)
            s = rt.tile([128, TT], F32, tag="s")
            nc.vector.tensor_reduce(s[:], ca_a3, mybir.AxisListType.X, ALU.add)
            om_all = rt.tile([128, TT], F32, tag="om", bufs=4)
            nc.vector.tensor_scalar(om_all[:], s[:], -1.0, 1.0, ALU.mult, ALU.add)
            selm = rt.tile([128, TT * E], F32, tag="selm")
            selm3 = selm[:].rearrange("p (j e) -> p j e", e=E)
            sel3 = sel_bc[:, None, :].broadcast_to([128, TT, E])
            nc.vector.tensor_tensor(selm3, ca_a3, sel3, op=ALU.mult)
            cac_all = rt.tile([128, TT], F32, tag="cac", bufs=4)
            nc.vector.tensor_reduce(
                cac_all[:], selm3, mybir.AxisListType.X, ALU.add
            )
            st_oneminus[c] = [om_all[:, j : j + 1] for j in range(TT)]
            return cac_all

        def route_meta(c, cac_all):
            """Rank selected tokens via PE cumsum, scatter (gid, ca) into
            the chunk's slot table, load back, and gather x rows."""
            # mask in f16 (counts <=512 are exact)
            maskh = meta.tile([128, TT], F16, tag="maskh")
            nc.vector.tensor_scalar(
                maskh[:], cac_all[:], 0.0, 1.0, ALU.is_gt, ALU.mult
            )
            ps_mx = ps_r.tile([128, 128], F32, tag="psx", bufs=2, name="ps_mx")
            ps_m = ps_mx[:, 0 : 2 * TT]
            nc.tensor.matmul(ps_m[:, 0:TT], utri[:], maskh[:],
                             start=True, stop=True)
            nc.tensor.matmul(ps_m[:, TT : 2 * TT], ones128[:], maskh[:],
                             start=True, stop=True)
            # global exclusive rank = (incol_incl - mask) + col-prefix totals
            slot_f = meta.tile([128, TT], F32, tag="slotf")
            nc.vector.tensor_tensor(slot_f[:], ps_m[:, 0:TT], maskh[:],
                                    op=ALU.subtract)
            tots = meta.tile([128, TT], F32, tag="tots")
            nc.vector.tensor_copy(tots[:], ps_m[:, TT : 2 * TT])
            c01 = meta.tile([128, 2], F32, tag="c01")
            nc.vector.tensor_tensor(c01[:, 0:1], tots[:, 0:1],
                                    tots[:, 1:2], op=ALU.add)
            nc.vector.tensor_tensor(c01[:, 1:2], c01[:, 0:1],
                                    tots[:, 2:3], op=ALU.add)
            nc.vector.tensor_tensor(slot_f[:, 1:2], slot_f[:, 1:2],
                                    tots[:, 0:1], op=ALU.add)
            nc.vector.tensor_tensor(slot_f[:, 2:3], slot_f[:, 2:3],
                                    c01[:, 0:1], op=ALU.add)
            nc.vector.tensor_tensor(slot_f[:, 3:4], slot_f[:, 3:4],
                                    c01[:, 1:2], op=ALU.add)
            # push pad slots out of range of the one-hot compare
            padp = meta.tile([128, TT], F32, tag="padp")
            nc.vector.tensor_scalar(padp[:], cac_all[:], 0.0, BIGF,
                                    ALU.is_le, ALU.mult)
            nc.vector.tensor_add(slot_f[:], slot_f[:], padp[:])
            # packed (gid+1, ca) columns per token tile
            mv8 = meta.tile([128, 2 * TT], F32, tag="mv8")
            mv83 = mv8[:].rearrange("p (j two) -> p j two", two=2)
            nc.vector.tensor_scalar(mv83[:, :, 0], iota4_f[:],
                                    float(c * CT + 1), None, ALU.add)
            nc.vector.tensor_copy(mv83[:, :, 1], cac_all[:])
            # one-hot invert: oh[j][p, s] = (slot[p, j] == s)
            ohs = []
            for j in range(TT):
                oh = meta.tile([128, CAPM], F32, tag="oh", bufs=4, name="oh")
                nc.vector.tensor_tensor(
                    oh[:], slot_f[:, j : j + 1].broadcast_to([128, CAPM]),
                    iota256_f[:], op=ALU.is_equal,
                )
                ohs.append(oh)
            metas, xgrows = [], []
            for st in range(NST):
                ps_sl = ps_r.tile([128, 128], F32, tag="psx", bufs=2,
                                  name="ps_sl")[:, 0:2]
                for j in range(TT):
                    nc.tensor.matmul(
                        ps_sl, ohs[j][:, st * 128 : (st + 1) * 128],
                        mv8[:, 2 * j : 2 * j + 2],
                        start=(j == 0), stop=(j == TT - 1),
                    )
                idsf = meta.tile([128, 2], F32, tag=f"idsf{st}", bufs=3)
                nc.vector.tensor_copy(idsf[:], ps_sl)
                # gid = ids - 1, pads (ids==0) -> huge positive
                padf = meta.tile([128, 1], F32, tag=f"padf{st}", bufs=3)
                nc.vector.tensor_scalar(padf[:], idsf[:, 0:1], 0.5, BIGF,
                                        ALU.is_le, ALU.mult)
                gidf = meta.tile([128, 1], F32, tag=f"gidf{st}", bufs=3)
                nc.vector.tensor_tensor(gidf[:], idsf[:, 0:1], padf[:],
                                        op=ALU.add)
                nc.vector.tensor_scalar(gidf[:], gidf[:], -1.0, None, ALU.add)
                gid_i = meta.tile([128, 1], I32, tag=f"gidi{st}", bufs=3)
                nc.vector.tensor_copy(gid_i[:], gidf[:])
                lid_f = meta.tile([128, 1], F32, tag=f"lidf{st}", bufs=3)
                nc.vector.tensor_scalar(lid_f[:], gidf[:],
                                        float(-c * CT), None, ALU.add)
                lid_i = meta.tile([128, 1], I32, tag=f"lidi{st}", bufs=3)
                nc.vector.tensor_copy(lid_i[:], lid_f[:])
                metas.append((gid_i, lid_i, idsf[:, 1:2]))
                xgr = xg.tile([128, D], F16, tag=f"xgr{st}")
                nc.gpsimd.indirect_dma_start(
                    out=xgr[:],
                    out_offset=None,
                    in_=xrows,
                    in_offset=bass.IndirectOffsetOnAxis(
                        ap=gid_i[:, 0:1], axis=0
                    ),
                    bounds_check=T - 1,
                    oob_is_err=False,
                )
                xgrows.append(xgr)
            st_smeta[c] = metas
            st_xgrows[c] = xgrows

        def route_transpose(c):
            """PE-transpose gathered rows into xgT k-tiles [128, CAP]."""
            xgrows = st_xgrows[c]
            xgT = []
            for k in range(KD):
                t = xg.tile([128, CAP], F16, tag=f"xgT{k}")
                xgT.append(t)
            for st in range(NST):
                w = 128 if st == 0 else CAP - 128
                for k in range(KD):
                    psx = ps_r.tile([128, 128], F16, tag="psx", bufs=2)
                    nc.tensor.transpose(
                        psx[:, 0:w],
                        xgrows[st][0:w, k * 128 : (k + 1) * 128],
                        ident16[0:w, 0:w],
                    )
                    if k % 2 == 0:
                        nc.scalar.activation(
                            xgT[k][:, st * 128 : st * 128 + w],
                            psx[:, 0:w], ACTF.Copy,
                        )
                    else:
                        nc.vector.tensor_copy(
                            xgT[k][:, st * 128 : st * 128 + w], psx[:, 0:w]
                        )
            st_xgT[c] = xgT

        def swiglu(psg, psu, width, tag):
            sig = work.tile([128, width], F32, tag="sig")
            nc.scalar.activation(sig[:], psg[:], ACTF.Sigmoid)
            sil = work.tile([128, width], F32, tag="sil")
            nc.vector.tensor_mul(sil[:], sig[:], psg[:])
            h = hp.tile([128, width], F16, tag=tag)
            nc.vector.tensor_mul(h[:], sil[:], psu[:])
            return h

        # ---- prologue: routing for chunk 0 ----------------------------
        Lt0 = route_mms(0)
        cac0 = route_combine(0, Lt0)
        route_meta(0, cac0)

        # chunks whose routing is computed inside each body
        ROUTE_IN_BODY = {0: [1, 2], 1: [3], 2: [], 3: []}

        # ---- main loop ------------------------------------------------
        rs_outs = []
        for c in range(CH):
            if c == 0:
                load_x(3)
            routes = ROUTE_IN_BODY[c]

            # shared gate/up (dense, full chunk; no routing dependency)
            xt = xt_all[c]
            hsc = []
            for f in range(KS):
                psg = ps_gu.tile([128, CT], F32, tag="psgu")
                for k in range(KD):
                    nc.tensor.matmul(
                        psg[:], wg[k][:, f * 128 : (f + 1) * 128], xt[k][:],
                        start=(k == 0), stop=(k == KD - 1),
                    )
                psu = ps_gu.tile([128, CT], F32, tag="psgu")
                for k in range(KD):
                    nc.tensor.matmul(
                        psu[:], wu[k][:, f * 128 : (f + 1) * 128], xt[k][:],
                        start=(k == 0), stop=(k == KD - 1),
                    )
                hsc.append(swiglu(psg, psu, CT, f"hs{f}"))

            # router matmuls + combine for lookahead chunks
            rstate = []
            for r in routes:
                Ltr = route_mms(r)
                cacr = route_combine(r, Ltr)
                rstate.append((r, cacr))

            # transpose gathered x for THIS chunk (gather already done)
            route_transpose(c)

            # shared down -> base rows (1-s)*shared_partial
            rs_in = dram.tile([CT, D], F16, tag=f"rsin{c}")
            oneminus = st_oneminus[c]
            for j in range(TT):
                contrib = work.tile([128, D], F16, tag="contrib")
                pss = [
                    ps_dn.tile([128, 512], F32, tag="psd", name=f"pss{dd}", bufs=2)
                    for dd in range(ND)
                ]
                for k in range(KS):
                    for dd in range(ND):
                        nc.tensor.matmul(
                            pss[dd][:],
                            hsc[k][:, j * 128 : (j + 1) * 128],
                            wd[k][:, dd * 512 : (dd + 1) * 512],
                            start=(k == 0), stop=(k == KS - 1),
                        )
                for dd in range(ND):
                    nc.vector.tensor_scalar(
                        contrib[:, dd * 512 : (dd + 1) * 512],
                        pss[dd][:], oneminus[j], None, ALU.mult,
                    )
                nc.sync.dma_start(
                    rs_in[j * 128 : (j + 1) * 128, :], contrib[:]
                )

            # routing metadata + x gathers for lookahead chunks
            for r, cacr in rstate:
                route_meta(r, cacr)

            # expert gate/up on routed tokens
            xgT = st_xgT[c]
            hc = []
            for f in range(KF):
                psg = ps_gu.tile([128, CT], F32, tag="psgu", name="psge_g")[:, 0:CAP]
                for k in range(KD):
                    nc.tensor.matmul(
                        psg[:], gq[k][:, f * 128 : (f + 1) * 128], xgT[k][:],
                        start=(k == 0), stop=(k == KD - 1),
                    )
                psu = ps_gu.tile([128, CT], F32, tag="psgu", name="psge_u")[:, 0:CAP]
                for k in range(KD):
                    nc.tensor.matmul(
                        psu[:], uq[k][:, f * 128 : (f + 1) * 128], xgT[k][:],
                        start=(k == 0), stop=(k == KD - 1),
                    )
                hc.append(swiglu(psg, psu, CAP, f"h{f}"))

            # expert down + scatter-ADD on top of base rows
            metas = st_smeta[c]
            for st in range(NST):
                w = 128 if st == 0 else CAP - 128
                gid_i, lid_i, ca_slot = metas[st]
                pse = [
                    ps_dn.tile([128, 512], F32, tag="psd", name=f"pse{dd}", bufs=2)
                    for dd in range(ND)
                ]
                for k in range(KF):
                    for dd in range(ND):
                        nc.tensor.matmul(
                            pse[dd][0:w, :],
                            hc[k][:, st * 128 : st * 128 + w],
                            dq[k][:, dd * 512 : (dd + 1) * 512],
                            start=(k == 0), stop=(k == KF - 1),
                        )
                ev = work.tile([128, D], F16, tag="ev")
                for dd in range(ND):
                    nc.vector.tensor_scalar(
                        ev[0:w, dd * 512 : (dd + 1) * 512],
                        pse[dd][0:w, :], ca_slot[0:w, :], None, ALU.mult,
                    )
                nc.gpsimd.indirect_dma_start(
                    out=rs_in[:],
                    out_offset=bass.IndirectOffsetOnAxis(
                        ap=lid_i[:, 0:1], axis=0
                    ),
                    in_=ev[:],
                    in_offset=None,
                    bounds_check=CT - 1,
                    oob_is_err=False,
                    compute_op=ALU.add,
                )

            # cross-core reduction (OUT copies all happen after the last
            # RS trigger so no copy-wait ever delays a trigger)
            rs_out = dram.tile([CT // NCORES, D], F16, tag=f"rsout{c}")
            rs_outs.append(rs_out)
            nc.gpsimd.collective_compute(
                "ReduceScatter", ALU.add,
                replica_groups=[list(range(NCORES))],
                ins=[rs_in.opt()], outs=[rs_out.opt()],
            )
            if c == CH - 1:
                for cc in range(CH):
                    nc.gpsimd.dma_start(
                        OUT[cc * 64 : (cc + 1) * 64, :], rs_outs[cc][:]
                    )

    nc.compile()
    return nc


def _prep_inputs(x, router_weight, sh_gate_w, sh_up_w, sh_down_w, gate_s,
                 up_s, down_s, alpha, gate_q, up_q, down_q):
    xf32 = np.ascontiguousarray(
        np.asarray(x, dtype=np.float32).reshape(T, D).T
    )
    xf = np.ascontiguousarray(xf32.astype(np.float16))
    xres = np.ascontiguousarray(
        (xf32 - xf.astype(np.float32)).astype(np.float16)
    )
    xrows = np.ascontiguousarray(xf.T)
    rw32 = np.asarray(router_weight, np.float32).T  # [D, E]
    rw_hi = rw32.astype(np.float16)
    rw_lo = (rw32 - rw_hi.astype(np.float32)).astype(np.float16)
    rwP = np.ascontiguousarray(np.concatenate([rw_hi, rw_lo], axis=1))
    in_maps = []
    for c in range(NCORES):
        gw = np.asarray(gate_q[c], np.float32) * np.asarray(
            gate_s[c], np.float32
        )[:, None]                                  # [DF_E, D]
        uw = np.asarray(up_q[c], np.float32) * np.asarray(
            up_s[c], np.float32
        )[:, None]                                  # [DF_E, D]
        dw = np.asarray(down_q[c], np.float32) * np.asarray(
            down_s[c], np.float32
        )[:, None]                                  # [D, DF_E]
        aux = np.zeros((128, 2 * E), np.float32)
        aux[:, 0:E] = np.asarray(alpha, np.float32)[None, :]
        aux[:, E + c] = 1.0
        in_maps.append(
            {
                "xT": xf,
                "xR": xres,
                "xrows": xrows,
                "rwP": rwP,
                "gqT": np.ascontiguousarray(gw.T.astype(np.float16)),
                "uqT": np.ascontiguousarray(uw.T.astype(np.float16)),
                "dqT": np.ascontiguousarray(dw.T.astype(np.float16)),
                "wgT": np.ascontiguousarray(
                    np.asarray(sh_gate_w[c * FS : (c + 1) * FS], np.float32)
                    .T.astype(np.float16)
                ),
                "wuT": np.ascontiguousarray(
                    np.asarray(sh_up_w[c * FS : (c + 1) * FS], np.float32)
                    .T.astype(np.float16)
                ),
                "wdT": np.ascontiguousarray(
                    np.asarray(sh_down_w[:, c * FS : (c + 1) * FS], np.float32)
                    .T.astype(np.float16)
                ),
                "aux": aux,
            }
        )
    return in_maps


def assemble(outs):
    """Reassemble per-core OUT tensors [256, D] into [B, S, D]."""
    out = np.empty((T, D), np.float32)
    for r in range(NCORES):
        o = np.asarray(outs[r])
        for c in range(CH):
            out[c * CT + r * 64 : c * CT + (r + 1) * 64] = (
                o[c * 64 : (c + 1) * 64]
            )
    return out.reshape(B, S, D)


def kernel(x, router_weight, sh_gate_w, sh_up_w, sh_down_w, gate_s, up_s,
           down_s, alpha, gate_q, up_q, down_q, top_k, **run_kwargs):
    assert int(top_k) == 2, "kernel compiled for top_k=2"
    assert tuple(np.shape(x)) == (B, S, D)

    if "nc" not in _CACHE:
        _CACHE["nc"] = _build()
    nc = _CACHE["nc"]

    in_maps = _prep_inputs(
        x, router_weight, sh_gate_w, sh_up_w, sh_down_w, gate_s, up_s,
        down_s, alpha, gate_q, up_q, down_q,
    )
    res = run_bass_kernel_spmd(
        nc, in_maps, core_ids=list(range(NCORES)), **run_kwargs
    )
    _CACHE["last_results"] = res

    outs = [res.results[r]["OUT"] for r in range(NCORES)]
    return assemble(outs).astype(np.asarray(x).dtype)


# revision 36
# speedup vs baseline: 1.0537x; 1.0537x over previous
"""Routed quantized MoE eval kernel for 8 Trainium2 NeuronCores.

Strategy (expert-parallel with true top-2 routing):
- Core c owns expert e=c. Expert weights are dequantized (scale-folded),
  cast to fp16 and transposed on the host at shard-prep time.
- Router runs dense on-device per 512-token chunk with split-precision
  fp16 matmuls (x16@[rw_hi|rw_lo] packed in one stationary + xres@rw_hi)
  accumulated in fp32 PSUM; top-2 + softmax + combine weights on DVE.
- ROUTED expert compute: for each chunk, tokens whose top-2 includes
  this core's expert are compacted via a PE-cumsum rank (triangular-ones
  matmul), their (token id, ca*alpha) pairs scattered into a DRAM slot
  table by indirect DMA (pads stay OOB), the x rows gathered by indirect
  DMA from a row-major fp16 copy of x, PE-transposed, and only those
  <=256 tokens (of 512) run the expert SwiGLU matmuls.
- Shared MLP sharded along DF_S (each core computes 256 of 2048 ffn
  rows) runs dense; each chunk's base rows (1-s)*shared_partial are
  written to a DRAM buffer, expert rows ca*alpha*eo are indirect-DMA
  scatter-ADDed on top (OOB pad slots skipped), then a chunked
  ReduceScatter sums across the 8 cores.
- Routing metadata for chunk c+1 is computed inside chunk c's body so
  all gathers complete before their expert matmuls need them.

Output identity:
  mixed = (1 - sum_e ca_e*alpha_e) * shared + sum_e ca_e*alpha_e * eo_e
"""

import numpy as np
from contextlib import ExitStack

import concourse.bass as bass
import concourse.tile as tile
from concourse import bacc, mybir
from concourse.bass_utils import run_bass_kernel_spmd

NCORES = 8
B, S, D = 2, 1024, 1024
T = B * S                      # 2048 tokens
DF_E, DF_S, E = 512, 2048, 8
FS = DF_S // NCORES            # 256 shared-ffn rows per core
CH = 4                         # token chunks
CT = T // CH                   # 512 tokens per chunk
TT = CT // 128                 # 4 token tiles per chunk
KD = D // 128                  # 8 k-tiles over hidden dim
KF = DF_E // 128               # 4 k-tiles over expert ffn dim
KS = FS // 128                 # 2 k-tiles over shared ffn shard
ND = D // 512                  # 2 output column slices
CAP = 192                      # routed-token capacity per (expert, chunk)
CAPM = 256                     # slot-metadata width (full 2x128)
NST = 2                        # slot tiles per chunk
BIGF = 4.0e6                   # OOB pusher for pad slots / ids

F16 = mybir.dt.float16
F32 = mybir.dt.float32
I32 = mybir.dt.int32
ACTF = mybir.ActivationFunctionType
ALU = mybir.AluOpType

_CACHE = {}


def _build():
    nc = bacc.Bacc(
        "TRN2", target_bir_lowering=False, debug=False, num_devices=NCORES
    )

    xT = nc.dram_tensor("xT", [D, T], F16, kind="ExternalInput").ap()
    xR = nc.dram_tensor("xR", [D, T], F16, kind="ExternalInput").ap()
    xrows = nc.dram_tensor("xrows", [T, D], F16, kind="ExternalInput").ap()
    # router weights split-precision: cols 0:E fp16-hi, E:2E fp16 residual
    rwP = nc.dram_tensor("rwP", [D, 2 * E], F16, kind="ExternalInput").ap()
    gqT = nc.dram_tensor("gqT", [D, DF_E], F16, kind="ExternalInput").ap()
    uqT = nc.dram_tensor("uqT", [D, DF_E], F16, kind="ExternalInput").ap()
    dqT = nc.dram_tensor("dqT", [DF_E, D], F16, kind="ExternalInput").ap()
    wgT = nc.dram_tensor("wgT", [D, FS], F16, kind="ExternalInput").ap()
    wuT = nc.dram_tensor("wuT", [D, FS], F16, kind="ExternalInput").ap()
    wdT = nc.dram_tensor("wdT", [FS, D], F16, kind="ExternalInput").ap()
    # aux[:, 0:8] = alpha broadcast, aux[:, 8:16] = onehot(expert) broadcast
    aux = nc.dram_tensor("aux", [128, 2 * E], F32, kind="ExternalInput").ap()
    OUT = nc.dram_tensor("OUT", [4 * 64, D], F16, kind="ExternalOutput").ap()

    with ExitStack() as ctx:
        tc = ctx.enter_context(tile.TileContext(nc))
        wres = ctx.enter_context(tc.tile_pool(name="wres", bufs=1))
        xs = ctx.enter_context(tc.tile_pool(name="xs", bufs=1))
        xg = ctx.enter_context(tc.tile_pool(name="xg", bufs=3))
        hp = ctx.enter_context(tc.tile_pool(name="hp", bufs=2))
        work = ctx.enter_context(tc.tile_pool(name="work", bufs=2))
        rt = ctx.enter_context(tc.tile_pool(name="rt", bufs=2))
        meta = ctx.enter_context(tc.tile_pool(name="meta", bufs=2))
        ps_gu = ctx.enter_context(tc.tile_pool(name="ps_gu", bufs=3, space="PSUM"))
        ps_dn = ctx.enter_context(tc.tile_pool(name="ps_dn", bufs=3, space="PSUM"))
        ps_r = ctx.enter_context(tc.tile_pool(name="ps_r", bufs=2, space="PSUM"))
        dram = ctx.enter_context(tc.tile_pool(name="dram", bufs=1, space="DRAM"))

        def load_rows(src, rows, cols, name):
            tiles = []
            r = src.rearrange("(k p) n -> k p n", p=128)
            for k in range(rows // 128):
                t = wres.tile([128, cols], src.dtype, tag=f"{name}{k}")
                nc.sync.dma_start(t[:], r[k])
                tiles.append(t)
            return tiles

        from concourse.masks import make_identity, make_upper_triangular

        # warmup ReduceScatter FIRST on the in-order gpsimd queue: the
        # ~54us ncfw cold-start begins at trigger time, so trigger ASAP
        wu_in = dram.tile([NCORES, 128], F16, tag="wuin")
        wu_out = dram.tile([1, 128], F16, tag="wuout")
        nc.sync.dma_start(wu_in[:], gqT[0:NCORES, 0:128])
        nc.gpsimd.collective_compute(
            "ReduceScatter",
            ALU.add,
            replica_groups=[list(range(NCORES))],
            ins=[wu_in.opt()],
            outs=[wu_out.opt()],
        )
        ident = wres.tile([128, 128], F32, tag="ident")
        make_identity(nc, ident[:])
        ident16 = wres.tile([128, 128], F16, tag="ident16")
        make_identity(nc, ident16[:])
        # inclusive upper-triangular ones (lhsT for partition cumsum)
        utri = wres.tile([128, 128], F16, tag="utri")
        make_upper_triangular(nc, utri[:], val=1.0, diag=True)
        ones128 = wres.tile([128, 128], F16, tag="ones128")
        nc.gpsimd.memset(ones128[:], 1.0)
        # iota4[p, j] = j*128 + p  (local token id within a chunk)
        iota4_i = wres.tile([128, TT], I32, tag="iota4i")
        nc.gpsimd.iota(iota4_i[:], pattern=[[128, TT]], base=0,
                       channel_multiplier=1)
        iota4_f = wres.tile([128, TT], F32, tag="iota4f")
        nc.vector.tensor_copy(iota4_f[:], iota4_i[:])
        # iota256[p, s] = s (slot index along free axis)
        iota256_i = wres.tile([128, CAPM], I32, tag="iota256i")
        nc.gpsimd.iota(iota256_i[:], pattern=[[1, CAPM]], base=0,
                       channel_multiplier=0)
        iota256_f = wres.tile([128, CAPM], F32, tag="iota256f")
        nc.vector.tensor_copy(iota256_f[:], iota256_i[:, 0:CAPM])

        # dummy matmuls during the initial DMA wait: warms the PE clock
        wu_ps = ps_r.tile([128, 128], F32, tag="psx", bufs=2)
        for _ in range(36):
            nc.tensor.matmul(wu_ps[:], ident[:], ident[:], start=True, stop=True)

        rw = load_rows(rwP, D, 2 * E, "rw")
        aux_sb = wres.tile([128, 2 * E], F32, tag="aux")
        nc.sync.dma_start(aux_sb[:], aux[:])
        alpha_bc = aux_sb[:, 0:E]
        sel_bc = aux_sb[:, E : 2 * E]

        xTr = xT.rearrange("(k p) t -> k p t", p=128)
        xRr = xR.rearrange("(k p) t -> k p t", p=128)

        xt_all = [None] * CH
        xr_all = [None] * CH

        def load_x(c):
            xt, xr = [], []
            for k in range(KD):
                t = xs.tile([128, CT], F16, tag=f"xt{c}_{k}")
                nc.sync.dma_start(t[:], xTr[k, :, c * CT : (c + 1) * CT])
                xt.append(t)
            for k in range(KD):
                t = xs.tile([128, CT], F16, tag=f"xr{c}_{k}")
                nc.sync.dma_start(t[:], xRr[k, :, c * CT : (c + 1) * CT])
                xr.append(t)
            xt_all[c], xr_all[c] = xt, xr

        load_x(0)
        wg = load_rows(wgT, D, FS, "wg")
        wu = load_rows(wuT, D, FS, "wu")
        load_x(1)
        load_x(2)
        gq = load_rows(gqT, D, DF_E, "gq")
        uq = load_rows(uqT, D, DF_E, "uq")
        wd = load_rows(wdT, FS, D, "wd")
        dq = load_rows(dqT, DF_E, D, "dq")

        # per-chunk routing state, filled by route_*()
        st_oneminus = [None] * CH   # [128,1] slices per j: (1 - s)
        st_smeta = [None] * CH      # list per st: (gid_i, lid_i, ca_slot)
        st_xgrows = [None] * CH     # gathered x rows tiles per st
        st_xgT = [None] * CH        # transposed gathered x per k-tile

        def route_mms(c):
            """Router matmuls for chunk c -> Lt [E, CT] f32 in SBUF."""
            xt, xr = xt_all[c], xr_all[c]
            ps_lg = ps_r.tile([40, CT], F32, tag="pslog", bufs=1, name="ps_lg")
            ps_lt = ps_lg[0 : 2 * E, :]
            ps_lr = ps_lg[32:40, :]
            for k in range(KD):
                nc.tensor.matmul(
                    ps_lt, rw[k][:, 0 : 2 * E], xt[k][:],
                    start=(k == 0), stop=(k == KD - 1),
                )
            for k in range(KD):
                nc.tensor.matmul(
                    ps_lr, rw[k][:, 0:E], xr[k][:],
                    start=(k == 0), stop=(k == KD - 1),
                )
            LtA = rt.tile([2 * E, CT], F32, tag="LtA")
            nc.vector.tensor_copy(LtA[:], ps_lg[0 : 2 * E, :])
            LtB = rt.tile([E, CT], F32, tag="LtB")
            nc.vector.tensor_copy(LtB[:], ps_lr)
            return (LtA, LtB)

        def route_combine(c, Lt):
            """Transpose logits, top-2 softmax, combine weights, and the
            rank/slot computation feeding the routed gather."""
            LtA, LtB = Lt
            ps_lx = ps_r.tile([128, 128], F32, tag="psx", bufs=2, name="ps_lx")
            ps_l = ps_lx[:, 0 : TT * 3 * E]
            for j in range(TT):
                nc.tensor.transpose(
                    ps_l[:, j * 3 * E : j * 3 * E + 2 * E],
                    LtA[:, j * 128 : (j + 1) * 128],
                    ident[0 : 2 * E, 0 : 2 * E],
                )
                nc.tensor.transpose(
                    ps_l[:, j * 3 * E + 2 * E : (j + 1) * 3 * E],
                    LtB[:, j * 128 : (j + 1) * 128],
                    ident[0:E, 0:E],
                )
            Lall = rt.tile([128, TT * 3 * E], F32, tag="Lall")
            nc.vector.tensor_copy(Lall[:], ps_l[:])
            va = Lall[:].rearrange("p (j x) -> p j x", x=3 * E)
            L = rt.tile([128, TT * E], F32, tag="L")
            L3 = L[:].rearrange("p (j e) -> p j e", e=E)
            nc.vector.tensor_tensor(L3, va[:, :, 0:E], va[:, :, E : 2 * E],
                                    op=ALU.add)
            nc.vector.tensor_tensor(L3, L3, va[:, :, 2 * E : 3 * E],
                                    op=ALU.add)

            def bc(t):  # [128, TT] -> [128, TT, E] free-axis broadcast
                return t[:, :, None].broadcast_to([128, TT, E])

            m1 = rt.tile([128, TT], F32, tag="m1")
            nc.vector.tensor_reduce(m1[:], L3, mybir.AxisListType.X, ALU.max)
            mask1 = rt.tile([128, TT * E], F32, tag="mask1")
            mask1_3 = mask1[:].rearrange("p (j e) -> p j e", e=E)
            nc.vector.tensor_tensor(mask1_3, L3, bc(m1), op=ALU.is_ge)
            L2 = rt.tile([128, TT * E], F32, tag="L2")
            nc.vector.scalar_tensor_tensor(
                L2[:], mask1[:], -1e30, L[:], ALU.mult, ALU.add
            )
            L2_3 = L2[:].rearrange("p (j e) -> p j e", e=E)
            m2 = rt.tile([128, TT], F32, tag="m2")
            nc.vector.tensor_reduce(m2[:], L2_3, mybir.AxisListType.X, ALU.max)
            mask2 = rt.tile([128, TT * E], F32, tag="mask2")
            mask2_3 = mask2[:].rearrange("p (j e) -> p j e", e=E)
            nc.vector.tensor_tensor(mask2_3, L2_3, bc(m2), op=ALU.is_ge)
            # softmax over {m1, m2}: w1 = sigmoid(m1 - m2), w2 = 1 - w1
            dlt = rt.tile([128, TT], F32, tag="dlt")
            nc.vector.tensor_sub(dlt[:], m1[:], m2[:])
            w1 = rt.tile([128, TT], F32, tag="w1")
            nc.scalar.activation(w1[:], dlt[:], ACTF.Sigmoid)
            w2 = rt.tile([128, TT], F32, tag="w2")
            nc.vector.tensor_scalar(w2[:], w1[:], -1.0, 1.0, ALU.mult, ALU.add)
            caw = rt.tile([128, TT * E], F32, tag="caw")
            caw3 = caw[:].rearrange("p (j e) -> p j e", e=E)
            nc.vector.tensor_tensor(caw3, mask2_3, bc(w2), op=ALU.mult)
            t1 = rt.tile([128, TT * E], F32, tag="t1")
            t1_3 = t1[:].rearrange("p (j e) -> p j e", e=E)
            nc.vector.tensor_tensor(t1_3, mask1_3, bc(w1), op=ALU.mult)
            nc.vector.tensor_add(caw[:], caw[:], t1[:])
            # scale by alpha (broadcast over token-tiles) and reduce
            ca_a = rt.tile([128, TT * E], F32, tag="ca_a")
            ca_a3 = ca_a[:].rearrange("p (j e) -> p j e", e=E)
            alpha3 = alpha_bc[:, None, :].broadcast_to([128, TT, E])
            nc.vector.tensor_tensor(ca_a3, caw3, alpha3, op=ALU.mult)
            s = rt.tile([128, TT], F32, tag="s")
            nc.vector.tensor_reduce(s[:], ca_a3, mybir.AxisListType.X, ALU.add)
            om_all = rt.tile([128, TT], F32, tag="om", bufs=4)
            nc.vector.tensor_scalar(om_all[:], s[:], -1.0, 1.0, ALU.mult, ALU.add)
            selm = rt.tile([128, TT * E], F32, tag="selm")
            selm3 = selm[:].rearrange("p (j e) -> p j e", e=E)
            sel3 = sel_bc[:, None, :].broadcast_to([128, TT, E])
            nc.vector.tensor_tensor(selm3, ca_a3, sel3, op=ALU.mult)
            cac_all = rt.tile([128, TT], F32, tag="cac", bufs=4)
            nc.vector.tensor_reduce(
                cac_all[:], selm3, mybir.AxisListType.X, ALU.add
            )
            st_oneminus[c] = [om_all[:, j : j + 1] for j in range(TT)]
            return cac_all

        def route_meta(c, cac_all):
            """Rank selected tokens via PE cumsum, scatter (gid, ca) into
            the chunk's slot table, load back, and gather x rows."""
            # mask in f16 (counts <=512 are exact)
            maskh = meta.tile([128, TT], F16, tag="maskh")
            nc.vector.tensor_scalar(
                maskh[:], cac_all[:], 0.0, 1.0, ALU.is_gt, ALU.mult
            )
            ps_mx = ps_r.tile([128, 128], F32, tag="psx", bufs=2, name="ps_mx")
            ps_m = ps_mx[:, 0 : 2 * TT]
            nc.tensor.matmul(ps_m[:, 0:TT], utri[:], maskh[:],
                             start=True, stop=True)
            nc.tensor.matmul(ps_m[:, TT : 2 * TT], ones128[:], maskh[:],
                             start=True, stop=True)
            # global exclusive rank = (incol_incl - mask) + col-prefix totals
            slot_f = meta.tile([128, TT], F32, tag="slotf")
            nc.vector.tensor_tensor(slot_f[:], ps_m[:, 0:TT], maskh[:],
                                    op=ALU.subtract)
            tots = meta.tile([128, TT], F32, tag="tots")
            nc.vector.tensor_copy(tots[:], ps_m[:, TT : 2 * TT])
            c01 = meta.tile([128, 2], F32, tag="c01")
            nc.vector.tensor_tensor(c01[:, 0:1], tots[:, 0:1],
                                    tots[:, 1:2], op=ALU.add)
            nc.vector.tensor_tensor(c01[:, 1:2], c01[:, 0:1],
                                    tots[:, 2:3], op=ALU.add)
            nc.vector.tensor_tensor(slot_f[:, 1:2], slot_f[:, 1:2],
                                    tots[:, 0:1], op=ALU.add)
            nc.vector.tensor_tensor(slot_f[:, 2:3], slot_f[:, 2:3],
                                    c01[:, 0:1], op=ALU.add)
            nc.vector.tensor_tensor(slot_f[:, 3:4], slot_f[:, 3:4],
                                    c01[:, 1:2], op=ALU.add)
            # push pad slots out of range of the one-hot compare
            padp = meta.tile([128, TT], F32, tag="padp")
            nc.vector.tensor_scalar(padp[:], cac_all[:], 0.0, BIGF,
                                    ALU.is_le, ALU.mult)
            nc.vector.tensor_add(slot_f[:], slot_f[:], padp[:])
            # packed (gid+1, ca) columns per token tile
            mv8 = meta.tile([128, 2 * TT], F32, tag="mv8")
            mv83 = mv8[:].rearrange("p (j two) -> p j two", two=2)
            nc.vector.tensor_scalar(mv83[:, :, 0], iota4_f[:],
                                    float(c * CT + 1), None, ALU.add)
            nc.vector.tensor_copy(mv83[:, :, 1], cac_all[:])
            # one-hot invert: oh[j][p, s] = (slot[p, j] == s)
            ohs = []
            for j in range(TT):
                oh = meta.tile([128, CAPM], F32, tag="oh", bufs=4, name="oh")
                nc.vector.tensor_tensor(
                    oh[:], slot_f[:, j : j + 1].broadcast_to([128, CAPM]),
                    iota256_f[:], op=ALU.is_equal,
                )
                ohs.append(oh)
            metas, xgrows = [], []
            for st in range(NST):
                ps_sl = ps_r.tile([128, 128], F32, tag="psx", bufs=2,
                                  name="ps_sl")[:, 0:2]
                for j in range(TT):
                    nc.tensor.matmul(
                        ps_sl, ohs[j][:, st * 128 : (st + 1) * 128],
                        mv8[:, 2 * j : 2 * j + 2],
                        start=(j == 0), stop=(j == TT - 1),
                    )
                idsf = meta.tile([128, 2], F32, tag=f"idsf{st}", bufs=3)
                nc.vector.tensor_copy(idsf[:], ps_sl)
                # gid = ids - 1, pads (ids==0) -> huge positive
                padf = meta.tile([128, 1], F32, tag=f"padf{st}", bufs=3)
                nc.vector.tensor_scalar(padf[:], idsf[:, 0:1], 0.5, BIGF,
                                        ALU.is_le, ALU.mult)
                gidf = meta.tile([128, 1], F32, tag=f"gidf{st}", bufs=3)
                nc.vector.tensor_tensor(gidf[:], idsf[:, 0:1], padf[:],
                                        op=ALU.add)
                nc.vector.tensor_scalar(gidf[:], gidf[:], -1.0, None, ALU.add)
                gid_i = meta.tile([128, 1], I32, tag=f"gidi{st}", bufs=3)
                nc.vector.tensor_copy(gid_i[:], gidf[:])
                lid_f = meta.tile([128, 1], F32, tag=f"lidf{st}", bufs=3)
                nc.vector.tensor_scalar(lid_f[:], gidf[:],
                                        float(-c * CT), None, ALU.add)
                lid_i = meta.tile([128, 1], I32, tag=f"lidi{st}", bufs=3)
                nc.vector.tensor_copy(lid_i[:], lid_f[:])
                metas.append((gid_i, lid_i, idsf[:, 1:2]))
                xgr = xg.tile([128, D], F16, tag=f"xgr{st}")
                nc.gpsimd.indirect_dma_start(
                    out=xgr[:],
                    out_offset=None,
                    in_=xrows,
                    in_offset=bass.IndirectOffsetOnAxis(
                        ap=gid_i[:, 0:1], axis=0
                    ),
                    bounds_check=T - 1,
                    oob_is_err=False,
                )
                xgrows.append(xgr)
            st_smeta[c] = metas
            st_xgrows[c] = xgrows

        def route_transpose(c):
            """PE-transpose gathered rows into xgT k-tiles [128, CAP]."""
            xgrows = st_xgrows[c]
            xgT = []
            for k in range(KD):
                t = xg.tile([128, CAP], F16, tag=f"xgT{k}")
                xgT.append(t)
            for st in range(NST):
                w = 128 if st == 0 else CAP - 128
                for k in range(KD):
                    psx = ps_r.tile([128, 128], F16, tag="psx", bufs=2)
                    nc.tensor.transpose(
                        psx[:, 0:w],
                        xgrows[st][0:w, k * 128 : (k + 1) * 128],
                        ident16[0:w, 0:w],
                    )
                    if k % 2 == 0:
                        nc.scalar.activation(
                            xgT[k][:, st * 128 : st * 128 + w],
                            psx[:, 0:w], ACTF.Copy,
                        )
                    else:
                        nc.vector.tensor_copy(
                            xgT[k][:, st * 128 : st * 128 + w], psx[:, 0:w]
                        )
            st_xgT[c] = xgT

        def swiglu(psg, psu, width, tag):
            sig = work.tile([128, width], F32, tag="sig")
            nc.scalar.activation(sig[:], psg[:], ACTF.Sigmoid)
            sil = work.tile([128, width], F32, tag="sil")
            nc.vector.tensor_mul(sil[:], sig[:], psg[:])
            h = hp.tile([128, width], F16, tag=tag)
            nc.vector.tensor_mul(h[:], sil[:], psu[:])
            return h

        # ---- prologue: routing for chunk 0 ----------------------------
        Lt0 = route_mms(0)
        cac0 = route_combine(0, Lt0)
        route_meta(0, cac0)

        # chunks whose routing is computed inside each body
        ROUTE_IN_BODY = {0: [1, 2], 1: [3], 2: [], 3: []}

        # ---- main loop ------------------------------------------------
        rs_outs = []
        for c in range(CH):
            if c == 0:
                load_x(3)
            routes = ROUTE_IN_BODY[c]

            # shared gate/up (dense, full chunk; no routing dependency)
            xt = xt_all[c]
            hsc = []
            for f in range(KS):
                psg = ps_gu.tile([128, CT], F32, tag="psgu")
                for k in range(KD):
                    nc.tensor.matmul(
                        psg[:], wg[k][:, f * 128 : (f + 1) * 128], xt[k][:],
                        start=(k == 0), stop=(k == KD - 1),
                    )
                psu = ps_gu.tile([128, CT], F32, tag="psgu")
                for k in range(KD):
                    nc.tensor.matmul(
                        psu[:], wu[k][:, f * 128 : (f + 1) * 128], xt[k][:],
                        start=(k == 0), stop=(k == KD - 1),
                    )
                hsc.append(swiglu(psg, psu, CT, f"hs{f}"))

            # router matmuls + combine for lookahead chunks
            rstate = []
            for r in routes:
                Ltr = route_mms(r)
                cacr = route_combine(r, Ltr)
                rstate.append((r, cacr))

            # transpose gathered x for THIS chunk (gather already done)
            route_transpose(c)

            # shared down -> base rows (1-s)*shared_partial
            rs_in = dram.tile([CT, D], F16, tag=f"rsin{c}")
            oneminus = st_oneminus[c]
            for j in range(TT):
                contrib = work.tile([128, D], F16, tag="contrib")
                pss = [
                    ps_dn.tile([128, 512], F32, tag="psd", name=f"pss{dd}", bufs=2)
                    for dd in range(ND)
                ]
                for k in range(KS):
                    for dd in range(ND):
                        nc.tensor.matmul(
                            pss[dd][:],
                            hsc[k][:, j * 128 : (j + 1) * 128],
                            wd[k][:, dd * 512 : (dd + 1) * 512],
                            start=(k == 0), stop=(k == KS - 1),
                        )
                for dd in range(ND):
                    nc.vector.tensor_scalar(
                        contrib[:, dd * 512 : (dd + 1) * 512],
                        pss[dd][:], oneminus[j], None, ALU.mult,
                    )
                nc.sync.dma_start(
                    rs_in[j * 128 : (j + 1) * 128, :], contrib[:]
                )

            # routing metadata + x gathers for lookahead chunks; in body 0
            # defer them past the RS trigger so RS(0) fires ~8us earlier
            # (the gathers still land ~20us before their transposes)
            if c != 0:
                for r, cacr in rstate:
                    route_meta(r, cacr)

            # expert gate/up on routed tokens
            xgT = st_xgT[c]
            hc = []
            for f in range(KF):
                psg = ps_gu.tile([128, CT], F32, tag="psgu", name="psge_g")[:, 0:CAP]
                for k in range(KD):
                    nc.tensor.matmul(
                        psg[:], gq[k][:, f * 128 : (f + 1) * 128], xgT[k][:],
                        start=(k == 0), stop=(k == KD - 1),
                    )
                psu = ps_gu.tile([128, CT], F32, tag="psgu", name="psge_u")[:, 0:CAP]
                for k in range(KD):
                    nc.tensor.matmul(
                        psu[:], uq[k][:, f * 128 : (f + 1) * 128], xgT[k][:],
                        start=(k == 0), stop=(k == KD - 1),
                    )
                hc.append(swiglu(psg, psu, CAP, f"h{f}"))

            # expert down + scatter-ADD on top of base rows
            metas = st_smeta[c]
            for st in range(NST):
                w = 128 if st == 0 else CAP - 128
                gid_i, lid_i, ca_slot = metas[st]
                pse = [
                    ps_dn.tile([128, 512], F32, tag="psd", name=f"pse{dd}", bufs=2)
                    for dd in range(ND)
                ]
                for k in range(KF):
                    for dd in range(ND):
                        nc.tensor.matmul(
                            pse[dd][0:w, :],
                            hc[k][:, st * 128 : st * 128 + w],
                            dq[k][:, dd * 512 : (dd + 1) * 512],
                            start=(k == 0), stop=(k == KF - 1),
                        )
                ev = work.tile([128, D], F16, tag="ev")
                for dd in range(ND):
                    nc.vector.tensor_scalar(
                        ev[0:w, dd * 512 : (dd + 1) * 512],
                        pse[dd][0:w, :], ca_slot[0:w, :], None, ALU.mult,
                    )
                nc.gpsimd.indirect_dma_start(
                    out=rs_in[:],
                    out_offset=bass.IndirectOffsetOnAxis(
                        ap=lid_i[:, 0:1], axis=0
                    ),
                    in_=ev[:],
                    in_offset=None,
                    bounds_check=CT - 1,
                    oob_is_err=False,
                    compute_op=ALU.add,
                )

            # cross-core reduction (OUT copies all happen after the last
            # RS trigger so no copy-wait ever delays a trigger)
            rs_out = dram.tile([CT // NCORES, D], F16, tag=f"rsout{c}")
            rs_outs.append(rs_out)
            nc.gpsimd.collective_compute(
                "ReduceScatter", ALU.add,
                replica_groups=[list(range(NCORES))],
                ins=[rs_in.opt()], outs=[rs_out.opt()],
            )
            if c == 0:
                for r, cacr in rstate:
                    route_meta(r, cacr)
            if c == CH - 1:
                for cc in range(CH):
                    nc.gpsimd.dma_start(
                        OUT[cc * 64 : (cc + 1) * 64, :], rs_outs[cc][:]
                    )

    nc.compile()
    return nc


def _prep_inputs(x, router_weight, sh_gate_w, sh_up_w, sh_down_w, gate_s,
                 up_s, down_s, alpha, gate_q, up_q, down_q):
    xf32 = np.ascontiguousarray(
        np.asarray(x, dtype=np.float32).reshape(T, D).T
    )
    xf = np.ascontiguousarray(xf32.astype(np.float16))
    xres = np.ascontiguousarray(
        (xf32 - xf.astype(np.float32)).astype(np.float16)
    )
    xrows = np.ascontiguousarray(xf.T)
    rw32 = np.asarray(router_weight, np.float32).T  # [D, E]
    rw_hi = rw32.astype(np.float16)
    rw_lo = (rw32 - rw_hi.astype(np.float32)).astype(np.float16)
    rwP = np.ascontiguousarray(np.concatenate([rw_hi, rw_lo], axis=1))
    in_maps = []
    for c in range(NCORES):
        gw = np.asarray(gate_q[c], np.float32) * np.asarray(
            gate_s[c], np.float32
        )[:, None]                                  # [DF_E, D]
        uw = np.asarray(up_q[c], np.float32) * np.asarray(
            up_s[c], np.float32
        )[:, None]                                  # [DF_E, D]
        dw = np.asarray(down_q[c], np.float32) * np.asarray(
            down_s[c], np.float32
        )[:, None]                                  # [D, DF_E]
        aux = np.zeros((128, 2 * E), np.float32)
        aux[:, 0:E] = np.asarray(alpha, np.float32)[None, :]
        aux[:, E + c] = 1.0
        in_maps.append(
            {
                "xT": xf,
                "xR": xres,
                "xrows": xrows,
                "rwP": rwP,
                "gqT": np.ascontiguousarray(gw.T.astype(np.float16)),
                "uqT": np.ascontiguousarray(uw.T.astype(np.float16)),
                "dqT": np.ascontiguousarray(dw.T.astype(np.float16)),
                "wgT": np.ascontiguousarray(
                    np.asarray(sh_gate_w[c * FS : (c + 1) * FS], np.float32)
                    .T.astype(np.float16)
                ),
                "wuT": np.ascontiguousarray(
                    np.asarray(sh_up_w[c * FS : (c + 1) * FS], np.float32)
                    .T.astype(np.float16)
                ),
                "wdT": np.ascontiguousarray(
                    np.asarray(sh_down_w[:, c * FS : (c + 1) * FS], np.float32)
                    .T.astype(np.float16)
                ),
                "aux": aux,
            }
        )
    return in_maps


def assemble(outs):
    """Reassemble per-core OUT tensors [256, D] into [B, S, D]."""
    out = np.empty((T, D), np.float32)
    for r in range(NCORES):
        o = np.asarray(outs[r])
        for c in range(CH):
            out[c * CT + r * 64 : c * CT + (r + 1) * 64] = (
                o[c * 64 : (c + 1) * 64]
            )
    return out.reshape(B, S, D)


def kernel(x, router_weight, sh_gate_w, sh_up_w, sh_down_w, gate_s, up_s,
           down_s, alpha, gate_q, up_q, down_q, top_k, **run_kwargs):
    assert int(top_k) == 2, "kernel compiled for top_k=2"
    assert tuple(np.shape(x)) == (B, S, D)

    if "nc" not in _CACHE:
        _CACHE["nc"] = _build()
    nc = _CACHE["nc"]

    in_maps = _prep_inputs(
        x, router_weight, sh_gate_w, sh_up_w, sh_down_w, gate_s, up_s,
        down_s, alpha, gate_q, up_q, down_q,
    )
    res = run_bass_kernel_spmd(
        nc, in_maps, core_ids=list(range(NCORES)), **run_kwargs
    )
    _CACHE["last_results"] = res

    outs = [res.results[r]["OUT"] for r in range(NCORES)]
    return assemble(outs).astype(np.asarray(x).dtype)
